# revision 11
# baseline (speedup 1.0000x reference)
"""Causal self-attention with rotary embeddings (B=2, T=2048, D=1024, H=16,
d_k=64) on 8 Trainium2 NeuronCores.

Sharding: core c handles batch b = c//4 and 4 heads (c%4)*4..+4 — data
parallel on B, tensor parallel on heads.  Each core computes its heads'
qkv projection, RoPE, causal attention, and a partial output projection
over its 256 attention channels; the host sums the 4 partials per batch.

Layout/perf notes:
  * everything is bf16 except PSUM accumulation, softmax scores (f32 in
    PSUM) and the reciprocal: x/Wqkv/Wout are cast host-side, q/k post
    rope, exp outputs, v, attn and the output partials are bf16
    (validated ~4e-3 rel err vs the 2e-2 gate).
  * q/k channels are de-interleaved host-side (RoPE pair -> half-split
    form) and packed 2 heads per 128-partition tile; the two heads'
    64-contraction score matmuls co-execute on separate PE row tiles.
  * RoPE swap (+/- sign) is a 128x128 permutation matmul on TensorE;
    cos/sin tables are natural scale, the 1/sqrt(d_k) folds into the
    exp's scale operand on the activation engine.
  * causal structure at 128-column granularity: score/av matmuls and
    exp only cover columns >= the key chunk's diagonal.  The intra-block
    mask of the diagonal 128x128 is applied on TensorE by accumulating
    -800 * triu(,1) into the scores PSUM (exp(0.125*(s-800)) == 0 in
    bf16), keeping the DVE out of the score->exp->av chain.
  * softmax denominator rides as a ones-column of v through the av
    matmul; reciprocal reads the PSUM row directly, gpsimd broadcasts,
    vector applies it during attnT eviction.
  * engine budget per core (warm): PE ~115us, ScalarE ~84us (exp is
    1 elem/lane/cycle @1.2GHz and irreducible), DVE ~70us.  The exp
    deficit inside attention waves is filled with qk/v projection
    chunks (waves 0-2) and all of the output projection (wave 3).
  * zero bias (the spec fills bqkv with zeros) skips the bias rank-1
    matmuls entirely; a with_bias program variant keeps generality.
"""

import sys

sys.path.insert(0, "/opt/trn_rl_repo")

import numpy as np
import ml_dtypes

import concourse.bacc as bacc
import concourse.tile as tile
from concourse import mybir
from concourse.bass_utils import run_bass_kernel_spmd

F32 = mybir.dt.float32
F32R = mybir.dt.float32r
BF16 = mybir.dt.bfloat16

B, T, D = 2, 2048, 1024
NH, DK = 16, 64
THETA = 10000.0
NCORES = 8
HEADS_PER_CORE = 4

TC512 = T // 512        # 4   i-chunks of 512
TC128 = T // 128        # 16  t/j-chunks of 128
KC = D // 128           # 8   d_model contraction chunks

MASK_BIAS = -800.0      # pre-scale; exp(0.125 * -800) flushes to 0

EXP = mybir.ActivationFunctionType.Exp
COPYF = mybir.ActivationFunctionType.Copy


def build_program(with_bias=False, debug=False):
    nc = bacc.Bacc("TRN2", target_bir_lowering=False, debug=False)

    XT = nc.dram_tensor("XT", [D + 1, T], BF16, kind="ExternalInput").ap()
    WQK = nc.dram_tensor("WQK", [D + 1, 512], BF16, kind="ExternalInput").ap()
    WV = nc.dram_tensor("WV", [D + 1, 256], BF16, kind="ExternalInput").ap()
    WOUT = nc.dram_tensor("WOUT", [256, D], BF16, kind="ExternalInput").ap()
    PT = nc.dram_tensor("PT", [128, 384], BF16, kind="ExternalInput").ap()
    CSQ = nc.dram_tensor("CSQ", [128, 2 * T], BF16, kind="ExternalInput").ap()
    ONES64 = nc.dram_tensor("ONES64", [1, 64], F32R, kind="ExternalInput").ap()
    OUT = nc.dram_tensor("OUT", [T, D], BF16, kind="ExternalOutput").ap()
    if debug:
        DBG_QKT = nc.dram_tensor("DBG_QKT", [128, 4 * T], BF16, kind="ExternalOutput").ap()
        DBG_V = nc.dram_tensor("DBG_V", [128, TC128 * 260], BF16, kind="ExternalOutput").ap()
        DBG_ATT = nc.dram_tensor("DBG_ATT", [128, 2 * T], BF16, kind="ExternalOutput").ap()

    with tile.TileContext(nc) as tc:
        with (
            tc.tile_pool(name="persist", bufs=1) as persist,
            tc.tile_pool(name="p1w", bufs=1) as p1w,
            tc.tile_pool(name="p1t", bufs=3) as p1t,
            tc.tile_pool(name="p2e", bufs=4) as p2e,
            tc.tile_pool(name="p2bc", bufs=2) as p2bc,
            tc.tile_pool(name="p2r", bufs=2) as p2r,
            tc.tile_pool(name="pj", bufs=2, space="PSUM") as pj,
            tc.tile_pool(name="sps", bufs=2, space="PSUM") as sps,
            tc.tile_pool(name="avps", bufs=2, space="PSUM") as avps,
        ):
            # ---- persistent tiles --------------------------------------
            qkT = persist.tile([128, 4 * T], BF16, tag="qkT")       # Qp0 Kp0 Qp1 Kp1
            v_sb = persist.tile([128, TC128 * 260], BF16, tag="v_sb")  # [jc, head, 64+1]
            attnT = persist.tile([128, 2 * T], BF16, tag="attnT")   # c-chunks x t
            wout_sb = persist.tile([128, 2 * D], BF16, tag="wout_sb")
            ones_sb = persist.tile([1, 64], F32R, tag="ones_sb")
            warm_sb = persist.tile([1, 8], F32, tag="warm_sb")

            x_sb = p1w.tile([128, KC * T], BF16, tag="x_sb")
            wqk_sb = p1w.tile([128, KC * 512], BF16, tag="wqk_sb")
            wv_sb = p1w.tile([128, KC * 256], BF16, tag="wv_sb")
            pt_sb = p1w.tile([128, 384], BF16, tag="pt_sb")
            psw_sb = pt_sb[:, 0:128]
            triu_sb = pt_sb[:, 128:256]
            negi_sb = pt_sb[:, 256:384]
            csq_sb = p1w.tile([128, 2 * T], BF16, tag="csq_sb")
            cq_sb = csq_sb[:, 0:T]
            sq_sb = csq_sb[:, T:2 * T]
            if with_bias:
                xlast = p1w.tile([1, T], BF16, tag="xlast")
                wqk_last = p1w.tile([1, 512], BF16, tag="wqk_last")
                wv_last = p1w.tile([1, 256], BF16, tag="wv_last")

            xt_src = XT[0:D, :].rearrange("(k p) t -> p k t", p=128)
            x_dst = x_sb[:].rearrange("p (k t) -> p k t", k=KC)

            # x block n: 2 batched triggers (k 0..3 / 4..7) on 2 queues
            def load_x_block(n, engines=(nc.sync, nc.gpsimd)):
                nsl = slice(n * 512, (n + 1) * 512)
                for half, eng in enumerate(engines):
                    ks = slice(half * 4, half * 4 + 4)
                    eng.dma_start(x_dst[:, ks, nsl], xt_src[:, ks, nsl])

            # ---- preamble loads ----------------------------------------
            # k-chunk granular, pipelined to match the PE's consumption
            # order (one (wqk_k, x_k) pair per ~0.85us), spread across 4
            # DMA-trigger queues (sync/gpsimd/scalar/vector) so triggers
            # don't serialize on one engine.
            wqk_dst = wqk_sb[:].rearrange("p (k c) -> p k c", k=KC)
            wqk_src = WQK[0:D, :].rearrange("(k p) c -> p k c", p=128)
            wv_dst = wv_sb[:].rearrange("p (k c) -> p k c", k=KC)
            wv_src = WV[0:D, :].rearrange("(k p) c -> p k c", p=128)

            # warm the exp table on ScalarE while DMA ramps (2.7us load)
            nc.vector.memset(warm_sb[:], 0.0)
            nc.scalar.activation(warm_sb[:], warm_sb[:], EXP, scale=0.125)

            # the (wqk_k, x_k@n0) pairs the first qk_proj consumes, round-
            # robined across all 3 trigger queues so no single queue's
            # bandwidth paces the critical path
            nc.scalar.dma_start(pt_sb[:], PT[:])
            nc.scalar.dma_start(csq_sb[:], CSQ[:])
            qs = [nc.sync, nc.gpsimd, nc.scalar]
            qi = 0
            for k in range(KC):
                qs[qi % 3].dma_start(wqk_dst[:, k], wqk_src[:, k]); qi += 1
                qs[qi % 3].dma_start(x_dst[:, k, 0:512], xt_src[:, k, 0:512]); qi += 1
            nc.sync.dma_start(wv_dst[:, 0:4], wv_src[:, 0:4])
            nc.gpsimd.dma_start(wv_dst[:, 4:8], wv_src[:, 4:8])
            nc.scalar.dma_start(ones_sb[:], ONES64[:])
            if with_bias:
                nc.gpsimd.dma_start(wqk_last[:], WQK[D:D + 1, :])
                nc.gpsimd.dma_start(xlast[:], XT[D:D + 1, :])
                nc.gpsimd.dma_start(wv_last[:], WV[D:D + 1, :])
            # x block n=1 early (consumed by qk_proj(.,1) inside wave 0)
            load_x_block(1, engines=(nc.sync, nc.scalar))

            # ones columns of v_aug: one strided memset
            v4 = v_sb[:].rearrange("p (jc h e) -> p jc h e", jc=TC128, h=4)
            nc.vector.memset(v4[:, :, :, 64:65], 1.0)

            # ---------------- building blocks ---------------------------
            def qk_proj_chunk(m, n):
                """project q/k m-chunk (128 channels) for t-chunk n (512), apply rope."""
                nsl = slice(n * 512, (n + 1) * 512)
                ps = pj.tile([128, 512], F32, tag="pj", name=f"psqk_{m}_{n}")
                for k in range(KC):
                    nc.tensor.matmul(
                        ps[:],
                        wqk_sb[:, k * 512 + m * 128:k * 512 + (m + 1) * 128],
                        x_sb[:, k * T + n * 512:k * T + (n + 1) * 512],
                        start=(k == 0), stop=(not with_bias and k == KC - 1),
                    )
                if with_bias:
                    nc.tensor.matmul(
                        ps[:], wqk_last[:, m * 128:(m + 1) * 128], xlast[:, nsl],
                        start=False, stop=True,
                    )
                tmp_s = p1t.tile([128, 512], BF16, tag="tmp_s", name=f"tmps_{m}_{n}")
                tmp_c = p1t.tile([128, 512], BF16, tag="tmp_c", name=f"tmpc_{m}_{n}")
                nc.vector.tensor_mul(tmp_s[:], ps[:], sq_sb[:, nsl])
                nc.vector.tensor_mul(tmp_c[:], ps[:], cq_sb[:, nsl])
                sw = pj.tile([128, 512], F32, tag="pj", name=f"sw_{m}_{n}")
                nc.tensor.matmul(sw[:], psw_sb, tmp_s[:], start=True, stop=True)
                nc.vector.tensor_add(qkT[:, m * T + n * 512:m * T + (n + 1) * 512], sw[:], tmp_c[:])

            def v_proj_chunk(tcc):
                tsl = slice(tcc * 128, (tcc + 1) * 128)
                psv = pj.tile([128, 256], F32, tag="pj", name=f"psv_{tcc}")
                for k in range(KC):
                    nc.tensor.matmul(
                        psv[:],
                        x_sb[:, k * T + tcc * 128:k * T + (tcc + 1) * 128],
                        wv_sb[:, k * 256:(k + 1) * 256],
                        start=(k == 0), stop=(not with_bias and k == KC - 1),
                    )
                if with_bias:
                    nc.tensor.matmul(psv[:], xlast[:, tsl], wv_last[:], start=False, stop=True)
                vdst = v_sb[:, tcc * 260:(tcc + 1) * 260].rearrange(
                    "p (h e) -> p h e", h=4)[:, :, 0:64]
                vsrc = psv[:].rearrange("p (h e) -> p h e", e=64)
                nc.vector.tensor_copy(vdst, vsrc)

            def attn_ic(p, ic, fillers=(), mid=None):
                """attention for head-pair p, query chunk ic (512 queries).
                fillers: callables run one per jc iteration (PE density).
                mid: when set (wave 0 only, njc <= e-pool bufs), all
                scores+exp are issued first, mid() runs, then the AV
                matmuls — so exp starts before mid()'s work (e.g. the
                v projections the AVs depend on) is even issued."""
                fillers = list(fillers)
                qof = (2 * p) * T
                kof = (2 * p + 1) * T
                njc = 4 * ic + 4
                av = [avps.tile([65, 512], F32, tag="av", name=f"av_{p}_{ic}_{i}") for i in range(2)]
                e_tiles = {}

                def scores_jc(jc):
                    rel = jc - 4 * ic
                    ls = 0 if rel < 0 else rel * 128
                    e_pair = p2e.tile([128, 1024], BF16, tag="e_t", name=f"e_{p}_{ic}_{jc}")
                    s_pair = sps.tile([128, 1024], F32, tag="s_ps", name=f"s_{p}_{ic}_{jc}")
                    for hh in range(2):
                        nc.tensor.matmul(
                            s_pair[:, hh * 512 + ls:(hh + 1) * 512],
                            qkT[hh * 64:hh * 64 + 64, kof + jc * 128:kof + (jc + 1) * 128],
                            qkT[hh * 64:hh * 64 + 64, qof + ic * 512 + ls:qof + (ic + 1) * 512],
                            start=True, stop=True,
                        )
                    if rel >= 0:
                        # intra-block causal mask: accumulate -800*triu(,1)
                        # into the diagonal 128x128 (TensorE, pre-exp)
                        for hh in range(2):
                            nc.tensor.matmul(
                                s_pair[:, hh * 512 + ls:hh * 512 + ls + 128],
                                triu_sb, negi_sb,
                                start=False, stop=True,
                                skip_group_check=True,
                            )
                    sv = s_pair[:].rearrange("p (h w) -> p h w", h=2)
                    ev = e_pair[:].rearrange("p (h w) -> p h w", h=2)
                    nc.scalar.activation(ev[:, :, ls:512], sv[:, :, ls:512], EXP, scale=0.125)
                    e_tiles[jc] = e_pair

                def av_jc(jc):
                    rel = jc - 4 * ic
                    ls = 0 if rel < 0 else rel * 128
                    e_pair = e_tiles.pop(jc)
                    for hh in range(2):
                        nc.tensor.matmul(
                            av[hh][:, ls:512],
                            v_sb[:, jc * 260 + (2 * p + hh) * 65:jc * 260 + (2 * p + hh) * 65 + 65],
                            e_pair[:, hh * 512 + ls:(hh + 1) * 512],
                            start=(jc == 0), stop=(jc == njc - 1),
                            skip_group_check=True,
                        )

                if mid is not None:
                    assert njc <= 4
                    for jc in range(njc):
                        scores_jc(jc)
                    mid()
                    for jc in range(njc):
                        av_jc(jc)
                        if fillers and (jc % max(1, njc // len(fillers)) == 0 or jc == njc - 1):
                            while fillers and len(fillers) > (njc - 1 - jc):
                                fillers.pop(0)()
                else:
                    for jc in range(njc):
                        scores_jc(jc)
                        av_jc(jc)
                        if fillers and (jc % max(1, njc // len(fillers)) == 0 or jc == njc - 1):
                            while fillers and len(fillers) > (njc - 1 - jc):
                                fillers.pop(0)()
                for hh in range(2):
                    head = 2 * p + hh
                    cof = (head // 2) * T
                    pof = (head % 2) * 64
                    den = p2r.tile([1, 512], F32, tag="den", name=f"den_{p}_{ic}_{hh}")
                    nc.vector.tensor_copy(den[:], av[hh][64:65, :])
                    rec = p2r.tile([1, 512], F32, tag="rec", name=f"rec_{p}_{ic}_{hh}")
                    nc.vector.reciprocal_approx_fast(rec[:], den[:])
                    dst = attnT[pof:pof + 64, cof + ic * 512:cof + (ic + 1) * 512]
                    bc_sb = p2bc.tile([64, 512], F32, tag="bc_sb", name=f"bc_{p}_{ic}_{hh}")
                    nc.gpsimd.partition_broadcast(bc_sb[:], rec[:], channels=64)
                    nc.vector.tensor_mul(dst, av[hh][0:64, :], bc_sb[:])

            def out_proj_chunk(tcc, dma_eng=None, split=False):
                """output projection for token chunk tcc.  split=True (tail):
                per-oc eviction on alternating engines + per-oc DMA on two
                queues so the last chunks drain with minimal serial chain."""
                tsl = slice(tcc * 128, (tcc + 1) * 128)
                po_sb = p1t.tile([128, 1024], BF16, tag="po_sb", name=f"po_sb_{tcc}")
                for oc in range(2):
                    po = pj.tile([128, 512], F32, tag="pj", name=f"po_{tcc}_{oc}")
                    for cc in range(2):
                        nc.tensor.matmul(
                            po[:],
                            attnT[:, cc * T + tcc * 128:cc * T + (tcc + 1) * 128],
                            wout_sb[:, cc * D + oc * 512:cc * D + (oc + 1) * 512],
                            start=(cc == 0), stop=(cc == 1),
                        )
                    osl = slice(oc * 512, (oc + 1) * 512)
                    if split:
                        ev = (nc.vector.tensor_copy, nc.scalar.copy)[oc]
                        ev(po_sb[:, osl], po[:])
                        (nc.sync, nc.gpsimd)[oc].dma_start(OUT[tsl, osl], po_sb[:, osl])
                    else:
                        nc.vector.tensor_copy(po_sb[:, osl], po[:])
                if not split:
                    (dma_eng or nc.sync).dma_start(OUT[tsl, :], po_sb[:])

            # ---------------- schedule: n-major waves -------------------
            # Wave n runs attention for query chunk n of both head pairs.
            # exp on ScalarE outweighs the attention matmuls 4:3, so each
            # wave carries PE filler: waves 0-2 the next wave's qk/v
            # projections, wave 3 the whole output projection (its waves'
            # attnT rows are complete by then).  Wave 0 starts as soon as
            # its own q/k chunks exist (m0/m1); its scores+exp are issued
            # before the v projections so ScalarE spins up ASAP.
            def load_wout():
                nc.scalar.dma_start(wout_sb[:, 0:D], WOUT[0:128, :])
                nc.scalar.dma_start(wout_sb[:, D:2 * D], WOUT[128:256, :])

            qk_proj_chunk(0, 0)
            qk_proj_chunk(1, 0)
            attn_ic(0, 0,
                    fillers=[lambda: qk_proj_chunk(2, 0), lambda: qk_proj_chunk(3, 0),
                             lambda: qk_proj_chunk(0, 1), lambda: qk_proj_chunk(1, 1)],
                    mid=lambda: [v_proj_chunk(t) for t in range(4)])
            attn_ic(1, 0,
                    fillers=[lambda: qk_proj_chunk(2, 1), lambda: qk_proj_chunk(3, 1),
                             lambda: v_proj_chunk(4), lambda: v_proj_chunk(5),
                             lambda: v_proj_chunk(6), lambda: v_proj_chunk(7),
                             lambda: load_x_block(2), load_wout])
            for n in range(1, TC512):
                fill0, fill1 = [], []
                if n < 3:
                    nx = n + 1
                    if nx + 1 < TC512:
                        fill0 += [lambda b=nx + 1: load_x_block(b)]
                    fill0 += [(lambda m=m: qk_proj_chunk(m, nx)) for m in range(4)]
                    fill0 += [(lambda t=t: v_proj_chunk(t)) for t in range(4 * nx, 4 * nx + 4)]
                if n == 3:
                    fill1 += [(lambda t=t, e=e: out_proj_chunk(t, e))
                              for t, e in zip(range(0, 12),
                                              [nc.sync, nc.gpsimd] * 6)]
                half = len(fill0) // 2
                attn_ic(0, n, fill0[:half] + fill1[:4])
                attn_ic(1, n, fill0[half:] + fill1[4:])
            for tcc in range(12, 16):
                out_proj_chunk(tcc, split=True)

            if debug:
                nc.sync.dma_start(DBG_QKT[:], qkT[:])
                nc.sync.dma_start(DBG_V[:], v_sb[:])
                nc.sync.dma_start(DBG_ATT[:], attnT[:])

    nc.compile()
    return nc


_DEINT = list(range(0, DK, 2)) + list(range(1, DK, 2))


def _rope_tables():
    j = np.arange(DK // 2, dtype=np.float64)
    inv_freq = THETA ** (-2.0 * j / DK)
    t = np.arange(T, dtype=np.float64)
    ang = t[None, :] * inv_freq[:, None]          # [32, T]
    ang = np.tile(ang, (4, 1))                    # [128, T]
    return np.cos(ang), np.sin(ang)


def _psw():
    M = np.zeros((128, 128), dtype=np.float32)
    for p in range(128):
        pm = p % 64
        if pm < 32:
            M[p, p + 32] = -1.0
        else:
            M[p, p - 32] = 1.0
    return np.ascontiguousarray(M.T)


def shard_inputs(x, Wqkv, bqkv, Wout, bout):
    bf = ml_dtypes.bfloat16
    x = np.asarray(x, dtype=np.float32)
    Wqkv = np.asarray(Wqkv, dtype=np.float32)
    bqkv = np.asarray(bqkv, dtype=np.float32)
    Wout = np.asarray(Wout, dtype=np.float32)

    cos_t, sin_t = _rope_tables()
    csq = np.ascontiguousarray(
        np.concatenate([cos_t, sin_t], axis=1)).astype(bf)   # [128, 2T]
    psw = _psw()
    # stationary strict-upper mask (lhsT[c,p] = 1 for key p > query c) and
    # the -800*I moving operand for the diagonal-block mask matmul
    triu = np.triu(np.ones((128, 128), dtype=np.float32), 1)
    negi = MASK_BIAS * np.eye(128, dtype=np.float32)
    pt = np.ascontiguousarray(np.concatenate([psw, triu, negi], axis=1)).astype(bf)
    ones64 = np.ones((1, 64), dtype=np.float32)

    Wfull = np.concatenate([Wqkv, bqkv[:, None]], axis=1)  # [3072, 1025]

    xt = {}
    for b in range(B):
        xt[b] = np.ascontiguousarray(
            np.concatenate([x[b].T, np.ones((1, T), np.float32)], axis=0)
        ).astype(bf)

    in_maps = []
    for c in range(NCORES):
        b = c // 4
        heads = [4 * (c % 4) + i for i in range(HEADS_PER_CORE)]
        # chunk order: [Qp0 | Kp0 | Qp1 | Kp1], each 128 rows (2 heads x 64)
        qk_rows = []
        for p in range(2):
            qrows, krows = [], []
            for h in (2 * p, 2 * p + 1):
                H = heads[h]
                qrows += [H * 192 + j for j in _DEINT]
                krows += [H * 192 + 64 + j for j in _DEINT]
            qk_rows += qrows + krows
        v_rows = []
        for h in range(4):
            H = heads[h]
            v_rows += [H * 192 + 128 + j for j in range(DK)]
        vch_out = []
        for h in range(4):
            H = heads[h]
            vch_out += [H * 64 + j for j in range(DK)]

        in_maps.append({
            "XT": xt[b],
            "WQK": np.ascontiguousarray(Wfull[qk_rows].T).astype(bf),
            "WV": np.ascontiguousarray(Wfull[v_rows].T).astype(bf),
            "WOUT": np.ascontiguousarray(Wout[:, vch_out].T).astype(bf),
            "PT": pt,
            "CSQ": csq,
            "ONES64": ones64,
        })
    return in_maps


_CACHED = {}


def _get_program(with_bias=False, debug=False):
    key = (bool(with_bias), bool(debug))
    if key not in _CACHED:
        _CACHED[key] = build_program(with_bias=with_bias, debug=debug)
    return _CACHED[key]


def run_cores(inputs, debug=False, trace=False, tmpdir=None):
    with_bias = bool(np.any(np.asarray(inputs["bqkv"], dtype=np.float32)))
    nc = _get_program(with_bias=with_bias, debug=debug)
    in_maps = shard_inputs(**inputs)
    res = run_bass_kernel_spmd(
        nc, in_maps, core_ids=list(range(NCORES)), trace=trace, tmpdir=tmpdir,
    )
    return res


def combine(results, bout):
    bout = np.asarray(bout, dtype=np.float32)
    out = np.empty((B, T, D), dtype=np.float32)
    for b in range(B):
        acc = results[4 * b]["OUT"].astype(np.float32)
        for c in range(4 * b + 1, 4 * b + 4):
            acc = acc + results[c]["OUT"].astype(np.float32)
        out[b] = acc + bout[None, :]
    return out


def kernel(x, Wqkv, bqkv, Wout, bout):
    res = run_cores(dict(x=x, Wqkv=Wqkv, bqkv=bqkv, Wout=Wout, bout=bout))
    return combine(res.results, bout)


# revision 15
# speedup vs baseline: 1.0517x; 1.0517x over previous
"""Causal self-attention with rotary embeddings (B=2, T=2048, D=1024, H=16,
d_k=64) on 8 Trainium2 NeuronCores.

Sharding: core c handles batch b = c//4 and 4 heads (c%4)*4..+4 — data
parallel on B, tensor parallel on heads.  Each core computes its heads'
qkv projection, RoPE, causal attention, and a partial output projection
over its 256 attention channels; the host sums the 4 partials per batch.

Layout/perf notes:
  * everything is bf16 except PSUM accumulation, softmax scores (f32 in
    PSUM) and the reciprocal: x/Wqkv/Wout are cast host-side, q/k post
    rope, exp outputs, v, attn and the output partials are bf16
    (validated ~4e-3 rel err vs the 2e-2 gate).
  * q/k channels are de-interleaved host-side (RoPE pair -> half-split
    form) and packed 2 heads per 128-partition tile; the two heads'
    64-contraction score matmuls co-execute on separate PE row tiles.
  * RoPE swap (+/- sign) is a 128x128 permutation matmul on TensorE;
    cos/sin tables are natural scale, the 1/sqrt(d_k) folds into the
    exp's scale operand on the activation engine.
  * causal structure at 128-column granularity: score/av matmuls and
    exp only cover columns >= the key chunk's diagonal.  The intra-block
    mask of the diagonal 128x128 is applied on TensorE by accumulating
    -800 * triu(,1) into the scores PSUM (exp(0.125*(s-800)) == 0 in
    bf16), keeping the DVE out of the score->exp->av chain.
  * softmax denominator rides as a ones-column of v through the av
    matmul; reciprocal reads the PSUM row directly, gpsimd broadcasts,
    vector applies it during attnT eviction.
  * engine budget per core (warm): PE ~115us, ScalarE ~84us (exp is
    1 elem/lane/cycle @1.2GHz and irreducible), DVE ~70us.  The exp
    deficit inside attention waves is filled with qk/v projection
    chunks (waves 0-2) and all of the output projection (wave 3).
  * zero bias (the spec fills bqkv with zeros) skips the bias rank-1
    matmuls entirely; a with_bias program variant keeps generality.
"""

import sys

sys.path.insert(0, "/opt/trn_rl_repo")

import numpy as np
import ml_dtypes

import concourse.bacc as bacc
import concourse.tile as tile
from concourse import mybir
from concourse.bass_utils import run_bass_kernel_spmd

F32 = mybir.dt.float32
F32R = mybir.dt.float32r
BF16 = mybir.dt.bfloat16

B, T, D = 2, 2048, 1024
NH, DK = 16, 64
THETA = 10000.0
NCORES = 8
HEADS_PER_CORE = 4

TC512 = T // 512        # 4   i-chunks of 512
TC128 = T // 128        # 16  t/j-chunks of 128
KC = D // 128           # 8   d_model contraction chunks

MASK_BIAS = -800.0      # pre-scale; exp(0.125 * -800) flushes to 0

EXP = mybir.ActivationFunctionType.Exp
COPYF = mybir.ActivationFunctionType.Copy


def build_program(with_bias=False, debug=False):
    nc = bacc.Bacc("TRN2", target_bir_lowering=False, debug=False)

    XT = nc.dram_tensor("XT", [D + 1, T], BF16, kind="ExternalInput").ap()
    WQK = nc.dram_tensor("WQK", [D + 1, 512], BF16, kind="ExternalInput").ap()
    WV = nc.dram_tensor("WV", [D + 1, 256], BF16, kind="ExternalInput").ap()
    WOUT = nc.dram_tensor("WOUT", [256, D], BF16, kind="ExternalInput").ap()
    PT = nc.dram_tensor("PT", [128, 384], BF16, kind="ExternalInput").ap()
    CSQ = nc.dram_tensor("CSQ", [128, 2 * T], BF16, kind="ExternalInput").ap()
    ONES64 = nc.dram_tensor("ONES64", [1, 64], F32R, kind="ExternalInput").ap()
    OUT = nc.dram_tensor("OUT", [T, D], BF16, kind="ExternalOutput").ap()
    if debug:
        DBG_QKT = nc.dram_tensor("DBG_QKT", [128, 4 * T], BF16, kind="ExternalOutput").ap()
        DBG_V = nc.dram_tensor("DBG_V", [128, TC128 * 260], BF16, kind="ExternalOutput").ap()
        DBG_ATT = nc.dram_tensor("DBG_ATT", [128, 2 * T], BF16, kind="ExternalOutput").ap()

    with tile.TileContext(nc) as tc:
        with (
            tc.tile_pool(name="persist", bufs=1) as persist,
            tc.tile_pool(name="p1w", bufs=1) as p1w,
            tc.tile_pool(name="p1t", bufs=3) as p1t,
            tc.tile_pool(name="p2e", bufs=4) as p2e,
            tc.tile_pool(name="p2bc", bufs=2) as p2bc,
            tc.tile_pool(name="p2r", bufs=2) as p2r,
            tc.tile_pool(name="pj", bufs=2, space="PSUM") as pj,
            tc.tile_pool(name="sps", bufs=2, space="PSUM") as sps,
            tc.tile_pool(name="avps", bufs=2, space="PSUM") as avps,
        ):
            # ---- persistent tiles --------------------------------------
            qkT = persist.tile([128, 4 * T], BF16, tag="qkT")       # Qp0 Kp0 Qp1 Kp1
            v_sb = persist.tile([128, TC128 * 260], BF16, tag="v_sb")  # [jc, head, 64+1]
            attnT = persist.tile([128, 2 * T], BF16, tag="attnT")   # c-chunks x t
            wout_sb = persist.tile([128, 2 * D], BF16, tag="wout_sb")
            ones_sb = persist.tile([1, 64], F32R, tag="ones_sb")
            warm_sb = persist.tile([1, 8], F32, tag="warm_sb")

            x_sb = p1w.tile([128, KC * T], BF16, tag="x_sb")
            wqk_sb = p1w.tile([128, KC * 512], BF16, tag="wqk_sb")
            wv_sb = p1w.tile([128, KC * 256], BF16, tag="wv_sb")
            pt_sb = p1w.tile([128, 384], BF16, tag="pt_sb")
            psw_sb = pt_sb[:, 0:128]
            triu_sb = pt_sb[:, 128:256]
            negi_sb = pt_sb[:, 256:384]
            csq_sb = p1w.tile([128, 2 * T], BF16, tag="csq_sb")
            cq_sb = csq_sb[:, 0:T]
            sq_sb = csq_sb[:, T:2 * T]
            if with_bias:
                xlast = p1w.tile([1, T], BF16, tag="xlast")
                wqk_last = p1w.tile([1, 512], BF16, tag="wqk_last")
                wv_last = p1w.tile([1, 256], BF16, tag="wv_last")

            xt_src = XT[0:D, :].rearrange("(k p) t -> p k t", p=128)
            x_dst = x_sb[:].rearrange("p (k t) -> p k t", k=KC)

            # x block n: 2 batched triggers (k 0..3 / 4..7) on 2 queues
            def load_x_block(n, engines=(nc.sync, nc.gpsimd)):
                nsl = slice(n * 512, (n + 1) * 512)
                for half, eng in enumerate(engines):
                    ks = slice(half * 4, half * 4 + 4)
                    eng.dma_start(x_dst[:, ks, nsl], xt_src[:, ks, nsl])

            # ---- preamble loads ----------------------------------------
            # k-chunk granular, pipelined to match the PE's consumption
            # order (one (wqk_k, x_k) pair per ~0.85us), spread across 4
            # DMA-trigger queues (sync/gpsimd/scalar/vector) so triggers
            # don't serialize on one engine.
            wqk_dst = wqk_sb[:].rearrange("p (k c) -> p k c", k=KC)
            wqk_src = WQK[0:D, :].rearrange("(k p) c -> p k c", p=128)
            wv_dst = wv_sb[:].rearrange("p (k c) -> p k c", k=KC)
            wv_src = WV[0:D, :].rearrange("(k p) c -> p k c", p=128)

            # warm the exp table on ScalarE while DMA ramps (2.7us load)
            nc.vector.memset(warm_sb[:], 0.0)
            nc.scalar.activation(warm_sb[:], warm_sb[:], EXP, scale=0.125)

            # the (wqk_k, x_k@n0) pairs the first qk_proj consumes: two
            # queues carry complementary halves in consumption order (the
            # DMA arbiter serves queues in unfair bursts, so a k-striped
            # round-robin stalls the strictly-ordered k-loop); pt/csq/wv
            # and x@n1 ride the third queue.
            nc.scalar.dma_start(pt_sb[:], PT[:])
            nc.scalar.dma_start(csq_sb[:], CSQ[:])
            for k in range(4):
                nc.sync.dma_start(wqk_dst[:, k], wqk_src[:, k])
                nc.gpsimd.dma_start(x_dst[:, k, 0:512], xt_src[:, k, 0:512])
            nc.scalar.dma_start(wv_dst[:, 0:4], wv_src[:, 0:4])
            nc.scalar.dma_start(wv_dst[:, 4:8], wv_src[:, 4:8])
            for k in range(4, 8):
                nc.sync.dma_start(x_dst[:, k, 0:512], xt_src[:, k, 0:512])
                nc.gpsimd.dma_start(wqk_dst[:, k], wqk_src[:, k])
            nc.scalar.dma_start(ones_sb[:], ONES64[:])
            if with_bias:
                nc.gpsimd.dma_start(wqk_last[:], WQK[D:D + 1, :])
                nc.gpsimd.dma_start(xlast[:], XT[D:D + 1, :])
                nc.gpsimd.dma_start(wv_last[:], WV[D:D + 1, :])
            # x block n=1 early (consumed by qk_proj(.,1) inside wave 0)
            load_x_block(1, engines=(nc.scalar, nc.scalar))

            # ones columns of v_aug: one strided memset
            v4 = v_sb[:].rearrange("p (jc h e) -> p jc h e", jc=TC128, h=4)
            nc.vector.memset(v4[:, :, :, 64:65], 1.0)

            # ---------------- building blocks ---------------------------
            def qk_proj_chunk(m, n):
                """project q/k m-chunk (128 channels) for t-chunk n (512), apply rope."""
                nsl = slice(n * 512, (n + 1) * 512)
                ps = pj.tile([128, 512], F32, tag="pj", name=f"psqk_{m}_{n}")
                for k in range(KC):
                    nc.tensor.matmul(
                        ps[:],
                        wqk_sb[:, k * 512 + m * 128:k * 512 + (m + 1) * 128],
                        x_sb[:, k * T + n * 512:k * T + (n + 1) * 512],
                        start=(k == 0), stop=(not with_bias and k == KC - 1),
                    )
                if with_bias:
                    nc.tensor.matmul(
                        ps[:], wqk_last[:, m * 128:(m + 1) * 128], xlast[:, nsl],
                        start=False, stop=True,
                    )
                tmp_s = p1t.tile([128, 512], BF16, tag="tmp_s", name=f"tmps_{m}_{n}")
                tmp_c = p1t.tile([128, 512], BF16, tag="tmp_c", name=f"tmpc_{m}_{n}")
                nc.vector.tensor_mul(tmp_s[:], ps[:], sq_sb[:, nsl])
                nc.vector.tensor_mul(tmp_c[:], ps[:], cq_sb[:, nsl])
                sw = pj.tile([128, 512], F32, tag="pj", name=f"sw_{m}_{n}")
                nc.tensor.matmul(sw[:], psw_sb, tmp_s[:], start=True, stop=True)
                nc.vector.tensor_add(qkT[:, m * T + n * 512:m * T + (n + 1) * 512], sw[:], tmp_c[:])

            def v_proj_chunk(tcc):
                tsl = slice(tcc * 128, (tcc + 1) * 128)
                psv = pj.tile([128, 256], F32, tag="pj", name=f"psv_{tcc}")
                for k in range(KC):
                    nc.tensor.matmul(
                        psv[:],
                        x_sb[:, k * T + tcc * 128:k * T + (tcc + 1) * 128],
                        wv_sb[:, k * 256:(k + 1) * 256],
                        start=(k == 0), stop=(not with_bias and k == KC - 1),
                    )
                if with_bias:
                    nc.tensor.matmul(psv[:], xlast[:, tsl], wv_last[:], start=False, stop=True)
                vdst = v_sb[:, tcc * 260:(tcc + 1) * 260].rearrange(
                    "p (h e) -> p h e", h=4)[:, :, 0:64]
                vsrc = psv[:].rearrange("p (h e) -> p h e", e=64)
                nc.vector.tensor_copy(vdst, vsrc)

            def attn_ic(p, ic, fillers=(), mid=None):
                """attention for head-pair p, query chunk ic (512 queries).
                fillers: callables run one per jc iteration (PE density).
                mid: when set (wave 0 only, njc <= e-pool bufs), all
                scores+exp are issued first, mid() runs, then the AV
                matmuls — so exp starts before mid()'s work (e.g. the
                v projections the AVs depend on) is even issued."""
                fillers = list(fillers)
                qof = (2 * p) * T
                kof = (2 * p + 1) * T
                njc = 4 * ic + 4
                av = [avps.tile([65, 512], F32, tag="av", name=f"av_{p}_{ic}_{i}") for i in range(2)]
                e_tiles = {}

                def scores_jc(jc):
                    rel = jc - 4 * ic
                    ls = 0 if rel < 0 else rel * 128
                    e_pair = p2e.tile([128, 1024], BF16, tag="e_t", name=f"e_{p}_{ic}_{jc}")
                    s_pair = sps.tile([128, 1024], F32, tag="s_ps", name=f"s_{p}_{ic}_{jc}")
                    for hh in range(2):
                        nc.tensor.matmul(
                            s_pair[:, hh * 512 + ls:(hh + 1) * 512],
                            qkT[hh * 64:hh * 64 + 64, kof + jc * 128:kof + (jc + 1) * 128],
                            qkT[hh * 64:hh * 64 + 64, qof + ic * 512 + ls:qof + (ic + 1) * 512],
                            start=True, stop=True,
                        )
                    if rel >= 0:
                        # intra-block causal mask: accumulate -800*triu(,1)
                        # into the diagonal 128x128 (TensorE, pre-exp)
                        for hh in range(2):
                            nc.tensor.matmul(
                                s_pair[:, hh * 512 + ls:hh * 512 + ls + 128],
                                triu_sb, negi_sb,
                                start=False, stop=True,
                                skip_group_check=True,
                            )
                    sv = s_pair[:].rearrange("p (h w) -> p h w", h=2)
                    ev = e_pair[:].rearrange("p (h w) -> p h w", h=2)
                    nc.scalar.activation(ev[:, :, ls:512], sv[:, :, ls:512], EXP, scale=0.125)
                    e_tiles[jc] = e_pair

                def av_jc(jc):
                    rel = jc - 4 * ic
                    ls = 0 if rel < 0 else rel * 128
                    e_pair = e_tiles.pop(jc)
                    for hh in range(2):
                        nc.tensor.matmul(
                            av[hh][:, ls:512],
                            v_sb[:, jc * 260 + (2 * p + hh) * 65:jc * 260 + (2 * p + hh) * 65 + 65],
                            e_pair[:, hh * 512 + ls:(hh + 1) * 512],
                            start=(jc == 0), stop=(jc == njc - 1),
                            skip_group_check=True,
                        )

                if mid is not None:
                    assert njc <= 4
                    for jc in range(njc):
                        scores_jc(jc)
                    mid()
                    for jc in range(njc):
                        av_jc(jc)
                        if fillers and (jc % max(1, njc // len(fillers)) == 0 or jc == njc - 1):
                            while fillers and len(fillers) > (njc - 1 - jc):
                                fillers.pop(0)()
                else:
                    # software-pipelined one jc ahead: scores(jc+1) issues
                    # before av(jc), so av's wait on exp(jc) is covered by
                    # independent PE work instead of an exposed stall.
                    scores_jc(0)
                    for jc in range(njc):
                        if jc + 1 < njc:
                            scores_jc(jc + 1)
                        av_jc(jc)
                        if fillers and (jc % max(1, njc // len(fillers)) == 0 or jc == njc - 1):
                            while fillers and len(fillers) > (njc - 1 - jc):
                                fillers.pop(0)()
                # normalization, phase-interleaved so the gpsimd broadcasts
                # overlap the vector ops instead of serializing per head
                dens, recs, bcs = [], [], []
                for hh in range(2):
                    den = p2r.tile([1, 512], F32, tag="den", name=f"den_{p}_{ic}_{hh}")
                    nc.vector.tensor_copy(den[:], av[hh][64:65, :])
                    dens.append(den)
                for hh in range(2):
                    rec = p2r.tile([1, 512], F32, tag="rec", name=f"rec_{p}_{ic}_{hh}")
                    nc.vector.reciprocal_approx_fast(rec[:], dens[hh][:])
                    recs.append(rec)
                    bc_sb = p2bc.tile([64, 512], F32, tag="bc_sb", name=f"bc_{p}_{ic}_{hh}")
                    nc.gpsimd.partition_broadcast(bc_sb[:], rec[:], channels=64)
                    bcs.append(bc_sb)
                for hh in range(2):
                    head = 2 * p + hh
                    cof = (head // 2) * T
                    pof = (head % 2) * 64
                    dst = attnT[pof:pof + 64, cof + ic * 512:cof + (ic + 1) * 512]
                    nc.vector.tensor_mul(dst, av[hh][0:64, :], bcs[hh][:])

            def out_proj_chunk(tcc, dma_eng=None, split=False):
                """output projection for token chunk tcc.  split=True (tail):
                per-oc eviction on alternating engines + per-oc DMA on two
                queues so the last chunks drain with minimal serial chain."""
                tsl = slice(tcc * 128, (tcc + 1) * 128)
                po_sb = p1t.tile([128, 1024], BF16, tag="po_sb", name=f"po_sb_{tcc}")
                for oc in range(2):
                    po = pj.tile([128, 512], F32, tag="pj", name=f"po_{tcc}_{oc}")
                    for cc in range(2):
                        nc.tensor.matmul(
                            po[:],
                            attnT[:, cc * T + tcc * 128:cc * T + (tcc + 1) * 128],
                            wout_sb[:, cc * D + oc * 512:cc * D + (oc + 1) * 512],
                            start=(cc == 0), stop=(cc == 1),
                        )
                    osl = slice(oc * 512, (oc + 1) * 512)
                    if split:
                        ev = (nc.vector.tensor_copy, nc.scalar.copy)[oc]
                        ev(po_sb[:, osl], po[:])
                        (nc.sync, nc.gpsimd)[oc].dma_start(OUT[tsl, osl], po_sb[:, osl])
                    else:
                        nc.vector.tensor_copy(po_sb[:, osl], po[:])
                if not split:
                    (dma_eng or nc.sync).dma_start(OUT[tsl, :], po_sb[:])

            # ---------------- schedule: n-major waves -------------------
            # Wave n runs attention for query chunk n of both head pairs.
            # exp on ScalarE outweighs the attention matmuls 4:3, so each
            # wave carries PE filler: waves 0-2 the next wave's qk/v
            # projections, wave 3 the whole output projection (its waves'
            # attnT rows are complete by then).  Wave 0 starts as soon as
            # its own q/k chunks exist (m0/m1); its scores+exp are issued
            # before the v projections so ScalarE spins up ASAP.
            def load_wout():
                nc.scalar.dma_start(wout_sb[:, 0:D], WOUT[0:128, :])
                nc.scalar.dma_start(wout_sb[:, D:2 * D], WOUT[128:256, :])

            qk_proj_chunk(0, 0)
            qk_proj_chunk(1, 0)
            attn_ic(0, 0,
                    fillers=[lambda: qk_proj_chunk(2, 0), lambda: qk_proj_chunk(3, 0),
                             lambda: qk_proj_chunk(0, 1), lambda: qk_proj_chunk(1, 1)],
                    mid=lambda: [v_proj_chunk(t) for t in range(4)])
            attn_ic(1, 0,
                    fillers=[lambda: qk_proj_chunk(2, 1), lambda: qk_proj_chunk(3, 1),
                             lambda: v_proj_chunk(4), lambda: v_proj_chunk(5),
                             lambda: v_proj_chunk(6), lambda: v_proj_chunk(7),
                             lambda: load_x_block(2), load_wout])
            for n in range(1, TC512):
                fill0, fill1 = [], []
                if n < 3:
                    nx = n + 1
                    if nx + 1 < TC512:
                        fill0 += [lambda b=nx + 1: load_x_block(b)]
                    fill0 += [(lambda m=m: qk_proj_chunk(m, nx)) for m in range(4)]
                    fill0 += [(lambda t=t: v_proj_chunk(t)) for t in range(4 * nx, 4 * nx + 4)]
                if n == 3:
                    fill1 += [(lambda t=t, e=e: out_proj_chunk(t, e))
                              for t, e in zip(range(0, 12),
                                              [nc.sync, nc.gpsimd] * 6)]
                half = len(fill0) // 2
                attn_ic(0, n, fill0[:half] + fill1[:8])
                attn_ic(1, n, fill0[half:] + fill1[8:])
            for tcc in range(12, 16):
                out_proj_chunk(tcc, split=True)

            if debug:
                nc.sync.dma_start(DBG_QKT[:], qkT[:])
                nc.sync.dma_start(DBG_V[:], v_sb[:])
                nc.sync.dma_start(DBG_ATT[:], attnT[:])

    nc.compile()
    return nc


_DEINT = list(range(0, DK, 2)) + list(range(1, DK, 2))


def _rope_tables():
    j = np.arange(DK // 2, dtype=np.float64)
    inv_freq = THETA ** (-2.0 * j / DK)
    t = np.arange(T, dtype=np.float64)
    ang = t[None, :] * inv_freq[:, None]          # [32, T]
    ang = np.tile(ang, (4, 1))                    # [128, T]
    return np.cos(ang), np.sin(ang)


def _psw():
    M = np.zeros((128, 128), dtype=np.float32)
    for p in range(128):
        pm = p % 64
        if pm < 32:
            M[p, p + 32] = -1.0
        else:
            M[p, p - 32] = 1.0
    return np.ascontiguousarray(M.T)


def shard_inputs(x, Wqkv, bqkv, Wout, bout):
    bf = ml_dtypes.bfloat16
    x = np.asarray(x, dtype=np.float32)
    Wqkv = np.asarray(Wqkv, dtype=np.float32)
    bqkv = np.asarray(bqkv, dtype=np.float32)
    Wout = np.asarray(Wout, dtype=np.float32)

    cos_t, sin_t = _rope_tables()
    csq = np.ascontiguousarray(
        np.concatenate([cos_t, sin_t], axis=1)).astype(bf)   # [128, 2T]
    psw = _psw()
    # stationary strict-upper mask (lhsT[c,p] = 1 for key p > query c) and
    # the -800*I moving operand for the diagonal-block mask matmul
    triu = np.triu(np.ones((128, 128), dtype=np.float32), 1)
    negi = MASK_BIAS * np.eye(128, dtype=np.float32)
    pt = np.ascontiguousarray(np.concatenate([psw, triu, negi], axis=1)).astype(bf)
    ones64 = np.ones((1, 64), dtype=np.float32)

    Wfull = np.concatenate([Wqkv, bqkv[:, None]], axis=1)  # [3072, 1025]

    xt = {}
    for b in range(B):
        xt[b] = np.ascontiguousarray(
            np.concatenate([x[b].T, np.ones((1, T), np.float32)], axis=0)
        ).astype(bf)

    in_maps = []
    for c in range(NCORES):
        b = c // 4
        heads = [4 * (c % 4) + i for i in range(HEADS_PER_CORE)]
        # chunk order: [Qp0 | Kp0 | Qp1 | Kp1], each 128 rows (2 heads x 64)
        qk_rows = []
        for p in range(2):
            qrows, krows = [], []
            for h in (2 * p, 2 * p + 1):
                H = heads[h]
                qrows += [H * 192 + j for j in _DEINT]
                krows += [H * 192 + 64 + j for j in _DEINT]
            qk_rows += qrows + krows
        v_rows = []
        for h in range(4):
            H = heads[h]
            v_rows += [H * 192 + 128 + j for j in range(DK)]
        vch_out = []
        for h in range(4):
            H = heads[h]
            vch_out += [H * 64 + j for j in range(DK)]

        in_maps.append({
            "XT": xt[b],
            "WQK": np.ascontiguousarray(Wfull[qk_rows].T).astype(bf),
            "WV": np.ascontiguousarray(Wfull[v_rows].T).astype(bf),
            "WOUT": np.ascontiguousarray(Wout[:, vch_out].T).astype(bf),
            "PT": pt,
            "CSQ": csq,
            "ONES64": ones64,
        })
    return in_maps


_CACHED = {}


def _get_program(with_bias=False, debug=False):
    key = (bool(with_bias), bool(debug))
    if key not in _CACHED:
        _CACHED[key] = build_program(with_bias=with_bias, debug=debug)
    return _CACHED[key]


def run_cores(inputs, debug=False, trace=False, tmpdir=None):
    with_bias = bool(np.any(np.asarray(inputs["bqkv"], dtype=np.float32)))
    nc = _get_program(with_bias=with_bias, debug=debug)
    in_maps = shard_inputs(**inputs)
    res = run_bass_kernel_spmd(
        nc, in_maps, core_ids=list(range(NCORES)), trace=trace, tmpdir=tmpdir,
    )
    return res


def combine(results, bout):
    bout = np.asarray(bout, dtype=np.float32)
    out = np.empty((B, T, D), dtype=np.float32)
    for b in range(B):
        acc = results[4 * b]["OUT"].astype(np.float32)
        for c in range(4 * b + 1, 4 * b + 4):
            acc = acc + results[c]["OUT"].astype(np.float32)
        out[b] = acc + bout[None, :]
    return out


def kernel(x, Wqkv, bqkv, Wout, bout):
    res = run_cores(dict(x=x, Wqkv=Wqkv, bqkv=bqkv, Wout=Wout, bout=bout))
    return combine(res.results, bout)


# revision 18
# speedup vs baseline: 1.0558x; 1.0039x over previous
"""Causal self-attention with rotary embeddings (B=2, T=2048, D=1024, H=16,
d_k=64) on 8 Trainium2 NeuronCores.

Sharding: core c handles batch b = c//4 and 4 heads (c%4)*4..+4 — data
parallel on B, tensor parallel on heads.  Each core computes its heads'
qkv projection, RoPE, causal attention, and a partial output projection
over its 256 attention channels; the host sums the 4 partials per batch.

Layout/perf notes:
  * everything is bf16 except PSUM accumulation, softmax scores (f32 in
    PSUM) and the reciprocal: x/Wqkv/Wout are cast host-side, q/k post
    rope, exp outputs, v, attn and the output partials are bf16
    (validated ~4e-3 rel err vs the 2e-2 gate).
  * q/k channels are de-interleaved host-side (RoPE pair -> half-split
    form) and packed 2 heads per 128-partition tile; the two heads'
    64-contraction score matmuls co-execute on separate PE row tiles.
  * RoPE swap (+/- sign) is a 128x128 permutation matmul on TensorE;
    cos/sin tables are natural scale, the 1/sqrt(d_k) folds into the
    exp's scale operand on the activation engine.
  * causal structure at 128-column granularity: score/av matmuls and
    exp only cover columns >= the key chunk's diagonal.  The intra-block
    mask of the diagonal 128x128 is applied on TensorE by accumulating
    -800 * triu(,1) into the scores PSUM (exp(0.125*(s-800)) == 0 in
    bf16), keeping the DVE out of the score->exp->av chain.
  * softmax denominator rides as a ones-column of v through the av
    matmul; reciprocal reads the PSUM row directly, gpsimd broadcasts,
    vector applies it during attnT eviction.
  * engine budget per core (warm): PE ~115us, ScalarE ~84us (exp is
    1 elem/lane/cycle @1.2GHz and irreducible), DVE ~70us.  The exp
    deficit inside attention waves is filled with qk/v projection
    chunks (waves 0-2) and all of the output projection (wave 3).
  * zero bias (the spec fills bqkv with zeros) skips the bias rank-1
    matmuls entirely; a with_bias program variant keeps generality.
"""

import sys

sys.path.insert(0, "/opt/trn_rl_repo")

import numpy as np
import ml_dtypes

import concourse.bacc as bacc
import concourse.tile as tile
from concourse import mybir
from concourse.bass_utils import run_bass_kernel_spmd

F32 = mybir.dt.float32
F32R = mybir.dt.float32r
BF16 = mybir.dt.bfloat16

B, T, D = 2, 2048, 1024
NH, DK = 16, 64
THETA = 10000.0
NCORES = 8
HEADS_PER_CORE = 4

TC512 = T // 512        # 4   i-chunks of 512
TC128 = T // 128        # 16  t/j-chunks of 128
KC = D // 128           # 8   d_model contraction chunks

MASK_BIAS = -800.0      # pre-scale; exp(0.125 * -800) flushes to 0

EXP = mybir.ActivationFunctionType.Exp
COPYF = mybir.ActivationFunctionType.Copy


def build_program(with_bias=False, debug=False):
    nc = bacc.Bacc("TRN2", target_bir_lowering=False, debug=False)

    XT = nc.dram_tensor("XT", [D + 1, T], BF16, kind="ExternalInput").ap()
    WQK = nc.dram_tensor("WQK", [D + 1, 512], BF16, kind="ExternalInput").ap()
    WV = nc.dram_tensor("WV", [D + 1, 256], BF16, kind="ExternalInput").ap()
    WOUT = nc.dram_tensor("WOUT", [256, D], BF16, kind="ExternalInput").ap()
    PT = nc.dram_tensor("PT", [128, 384], BF16, kind="ExternalInput").ap()
    CSQ = nc.dram_tensor("CSQ", [128, 2 * T], BF16, kind="ExternalInput").ap()
    ONES64 = nc.dram_tensor("ONES64", [1, 64], F32R, kind="ExternalInput").ap()
    OUT = nc.dram_tensor("OUT", [T, D], BF16, kind="ExternalOutput").ap()
    if debug:
        DBG_QKT = nc.dram_tensor("DBG_QKT", [128, 4 * T], BF16, kind="ExternalOutput").ap()
        DBG_V = nc.dram_tensor("DBG_V", [128, TC128 * 260], BF16, kind="ExternalOutput").ap()
        DBG_ATT = nc.dram_tensor("DBG_ATT", [128, 2 * T], BF16, kind="ExternalOutput").ap()

    with tile.TileContext(nc) as tc:
        with (
            tc.tile_pool(name="persist", bufs=1) as persist,
            tc.tile_pool(name="p1w", bufs=1) as p1w,
            tc.tile_pool(name="p1t", bufs=3) as p1t,
            tc.tile_pool(name="p2e", bufs=4) as p2e,
            tc.tile_pool(name="p2bc", bufs=2) as p2bc,
            tc.tile_pool(name="p2r", bufs=2) as p2r,
            tc.tile_pool(name="pj", bufs=2, space="PSUM") as pj,
            tc.tile_pool(name="sps", bufs=2, space="PSUM") as sps,
            tc.tile_pool(name="avps", bufs=2, space="PSUM") as avps,
        ):
            # ---- persistent tiles --------------------------------------
            qkT = persist.tile([128, 4 * T], BF16, tag="qkT")       # Qp0 Kp0 Qp1 Kp1
            v_sb = persist.tile([128, TC128 * 260], BF16, tag="v_sb")  # [jc, head, 64+1]
            attnT = persist.tile([128, 2 * T], BF16, tag="attnT")   # c-chunks x t
            wout_sb = persist.tile([128, 2 * D], BF16, tag="wout_sb")
            ones_sb = persist.tile([1, 64], F32R, tag="ones_sb")
            warm_sb = persist.tile([1, 8], F32, tag="warm_sb")

            x_sb = p1w.tile([128, KC * T], BF16, tag="x_sb")
            wqk_sb = p1w.tile([128, KC * 512], BF16, tag="wqk_sb")
            wv_sb = p1w.tile([128, KC * 256], BF16, tag="wv_sb")
            pt_sb = p1w.tile([128, 384], BF16, tag="pt_sb")
            psw_sb = pt_sb[:, 0:128]
            triu_sb = pt_sb[:, 128:256]
            negi_sb = pt_sb[:, 256:384]
            csq_sb = p1w.tile([128, 2 * T], BF16, tag="csq_sb")
            cq_sb = csq_sb[:, 0:T]
            sq_sb = csq_sb[:, T:2 * T]
            if with_bias:
                xlast = p1w.tile([1, T], BF16, tag="xlast")
                wqk_last = p1w.tile([1, 512], BF16, tag="wqk_last")
                wv_last = p1w.tile([1, 256], BF16, tag="wv_last")

            xt_src = XT[0:D, :].rearrange("(k p) t -> p k t", p=128)
            x_dst = x_sb[:].rearrange("p (k t) -> p k t", k=KC)

            # x block n: 2 batched triggers (k 0..3 / 4..7) on 2 queues
            def load_x_block(n, engines=(nc.sync, nc.gpsimd)):
                nsl = slice(n * 512, (n + 1) * 512)
                for half, eng in enumerate(engines):
                    ks = slice(half * 4, half * 4 + 4)
                    eng.dma_start(x_dst[:, ks, nsl], xt_src[:, ks, nsl])

            # ---- preamble loads ----------------------------------------
            # k-chunk granular, pipelined to match the PE's consumption
            # order (one (wqk_k, x_k) pair per ~0.85us), spread across 4
            # DMA-trigger queues (sync/gpsimd/scalar/vector) so triggers
            # don't serialize on one engine.
            wqk_dst = wqk_sb[:].rearrange("p (k c) -> p k c", k=KC)
            wqk_src = WQK[0:D, :].rearrange("(k p) c -> p k c", p=128)
            wv_dst = wv_sb[:].rearrange("p (k c) -> p k c", k=KC)
            wv_src = WV[0:D, :].rearrange("(k p) c -> p k c", p=128)

            # warm the exp table on ScalarE while DMA ramps (2.7us load)
            nc.vector.memset(warm_sb[:], 0.0)
            nc.scalar.activation(warm_sb[:], warm_sb[:], EXP, scale=0.125)

            # Preamble loads use few BIG triggers: per-trigger latency
            # (~2us descriptor-gen + doorbell) caps a queue at ~65GB/s on
            # 131KB chunks, so half-tensor (0.5MB) transfers are what let
            # the critical 2.1MB land in <10us.  Two queues carry
            # complementary halves in consumption order; pt/csq/wv ride
            # the third.
            nc.scalar.dma_start(pt_sb[:], PT[:])
            nc.scalar.dma_start(csq_sb[:], CSQ[:])
            nc.sync.dma_start(wqk_dst[:, 0:4], wqk_src[:, 0:4])
            nc.gpsimd.dma_start(x_dst[:, 0:4, 0:512], xt_src[:, 0:4, 0:512])
            nc.sync.dma_start(x_dst[:, 4:8, 0:512], xt_src[:, 4:8, 0:512])
            nc.gpsimd.dma_start(wqk_dst[:, 4:8], wqk_src[:, 4:8])
            nc.scalar.dma_start(wv_dst[:, 0:8], wv_src[:, 0:8])
            nc.scalar.dma_start(ones_sb[:], ONES64[:])
            if with_bias:
                nc.gpsimd.dma_start(wqk_last[:], WQK[D:D + 1, :])
                nc.gpsimd.dma_start(xlast[:], XT[D:D + 1, :])
                nc.gpsimd.dma_start(wv_last[:], WV[D:D + 1, :])
            # x block n=1 early (consumed by qk_proj(.,1) inside wave 0)
            load_x_block(1, engines=(nc.sync, nc.gpsimd))

            # ones columns of v_aug: one strided memset
            v4 = v_sb[:].rearrange("p (jc h e) -> p jc h e", jc=TC128, h=4)
            nc.vector.memset(v4[:, :, :, 64:65], 1.0)

            # ---------------- building blocks ---------------------------
            def qk_proj_chunk(m, n):
                """project q/k m-chunk (128 channels) for t-chunk n (512), apply rope."""
                nsl = slice(n * 512, (n + 1) * 512)
                ps = pj.tile([128, 512], F32, tag="pj", name=f"psqk_{m}_{n}")
                for k in range(KC):
                    nc.tensor.matmul(
                        ps[:],
                        wqk_sb[:, k * 512 + m * 128:k * 512 + (m + 1) * 128],
                        x_sb[:, k * T + n * 512:k * T + (n + 1) * 512],
                        start=(k == 0), stop=(not with_bias and k == KC - 1),
                    )
                if with_bias:
                    nc.tensor.matmul(
                        ps[:], wqk_last[:, m * 128:(m + 1) * 128], xlast[:, nsl],
                        start=False, stop=True,
                    )
                tmp_s = p1t.tile([128, 512], BF16, tag="tmp_s", name=f"tmps_{m}_{n}")
                tmp_c = p1t.tile([128, 512], BF16, tag="tmp_c", name=f"tmpc_{m}_{n}")
                nc.vector.tensor_mul(tmp_s[:], ps[:], sq_sb[:, nsl])
                nc.vector.tensor_mul(tmp_c[:], ps[:], cq_sb[:, nsl])
                sw = pj.tile([128, 512], F32, tag="pj", name=f"sw_{m}_{n}")
                nc.tensor.matmul(sw[:], psw_sb, tmp_s[:], start=True, stop=True)
                nc.vector.tensor_add(qkT[:, m * T + n * 512:m * T + (n + 1) * 512], sw[:], tmp_c[:])

            def v_proj_chunk(tcc):
                tsl = slice(tcc * 128, (tcc + 1) * 128)
                psv = pj.tile([128, 256], F32, tag="pj", name=f"psv_{tcc}")
                for k in range(KC):
                    nc.tensor.matmul(
                        psv[:],
                        x_sb[:, k * T + tcc * 128:k * T + (tcc + 1) * 128],
                        wv_sb[:, k * 256:(k + 1) * 256],
                        start=(k == 0), stop=(not with_bias and k == KC - 1),
                    )
                if with_bias:
                    nc.tensor.matmul(psv[:], xlast[:, tsl], wv_last[:], start=False, stop=True)
                vdst = v_sb[:, tcc * 260:(tcc + 1) * 260].rearrange(
                    "p (h e) -> p h e", h=4)[:, :, 0:64]
                vsrc = psv[:].rearrange("p (h e) -> p h e", e=64)
                # evict on ScalarE for the early chunks (ScalarE idles
                # before/between the small waves; the DVE queue is what
                # gates the next wave's rope adds)
                if tcc < 12:
                    nc.scalar.copy(vdst, vsrc)
                else:
                    nc.vector.tensor_copy(vdst, vsrc)

            def attn_ic(p, ic, fillers=(), mid=None):
                """attention for head-pair p, query chunk ic (512 queries).
                fillers: callables run one per jc iteration (PE density).
                mid: when set (wave 0 only, njc <= e-pool bufs), all
                scores+exp are issued first, mid() runs, then the AV
                matmuls — so exp starts before mid()'s work (e.g. the
                v projections the AVs depend on) is even issued."""
                fillers = list(fillers)
                qof = (2 * p) * T
                kof = (2 * p + 1) * T
                njc = 4 * ic + 4
                av = [avps.tile([65, 512], F32, tag="av", name=f"av_{p}_{ic}_{i}") for i in range(2)]
                e_tiles = {}

                def scores_jc(jc):
                    rel = jc - 4 * ic
                    ls = 0 if rel < 0 else rel * 128
                    e_pair = p2e.tile([128, 1024], BF16, tag="e_t", name=f"e_{p}_{ic}_{jc}")
                    s_pair = sps.tile([128, 1024], F32, tag="s_ps", name=f"s_{p}_{ic}_{jc}")
                    for hh in range(2):
                        nc.tensor.matmul(
                            s_pair[:, hh * 512 + ls:(hh + 1) * 512],
                            qkT[hh * 64:hh * 64 + 64, kof + jc * 128:kof + (jc + 1) * 128],
                            qkT[hh * 64:hh * 64 + 64, qof + ic * 512 + ls:qof + (ic + 1) * 512],
                            start=True, stop=True,
                        )
                    if rel >= 0:
                        # intra-block causal mask: accumulate -800*triu(,1)
                        # into the diagonal 128x128 (TensorE, pre-exp)
                        for hh in range(2):
                            nc.tensor.matmul(
                                s_pair[:, hh * 512 + ls:hh * 512 + ls + 128],
                                triu_sb, negi_sb,
                                start=False, stop=True,
                                skip_group_check=True,
                            )
                    sv = s_pair[:].rearrange("p (h w) -> p h w", h=2)
                    ev = e_pair[:].rearrange("p (h w) -> p h w", h=2)
                    nc.scalar.activation(ev[:, :, ls:512], sv[:, :, ls:512], EXP, scale=0.125)
                    e_tiles[jc] = e_pair

                def av_jc(jc):
                    rel = jc - 4 * ic
                    ls = 0 if rel < 0 else rel * 128
                    e_pair = e_tiles.pop(jc)
                    for hh in range(2):
                        nc.tensor.matmul(
                            av[hh][:, ls:512],
                            v_sb[:, jc * 260 + (2 * p + hh) * 65:jc * 260 + (2 * p + hh) * 65 + 65],
                            e_pair[:, hh * 512 + ls:(hh + 1) * 512],
                            start=(jc == 0), stop=(jc == njc - 1),
                            skip_group_check=True,
                        )

                if mid is not None:
                    assert njc <= 4
                    for jc in range(njc):
                        scores_jc(jc)
                    mid()
                    for jc in range(njc):
                        av_jc(jc)
                        if fillers and (jc % max(1, njc // len(fillers)) == 0 or jc == njc - 1):
                            while fillers and len(fillers) > (njc - 1 - jc):
                                fillers.pop(0)()
                else:
                    # software-pipelined one jc ahead: scores(jc+1) issues
                    # before av(jc), so av's wait on exp(jc) is covered by
                    # independent PE work instead of an exposed stall.
                    scores_jc(0)
                    for jc in range(njc):
                        if jc + 1 < njc:
                            scores_jc(jc + 1)
                        av_jc(jc)
                        if fillers and (jc % max(1, njc // len(fillers)) == 0 or jc == njc - 1):
                            while fillers and len(fillers) > (njc - 1 - jc):
                                fillers.pop(0)()
                # normalization, phase-interleaved so the gpsimd broadcasts
                # overlap the vector ops instead of serializing per head
                dens, recs, bcs = [], [], []
                for hh in range(2):
                    den = p2r.tile([1, 512], F32, tag="den", name=f"den_{p}_{ic}_{hh}")
                    if ic < 2:
                        nc.scalar.copy(den[:], av[hh][64:65, :])
                    else:
                        nc.vector.tensor_copy(den[:], av[hh][64:65, :])
                    dens.append(den)
                for hh in range(2):
                    rec = p2r.tile([1, 512], F32, tag="rec", name=f"rec_{p}_{ic}_{hh}")
                    nc.vector.reciprocal_approx_fast(rec[:], dens[hh][:])
                    recs.append(rec)
                    bc_sb = p2bc.tile([64, 512], F32, tag="bc_sb", name=f"bc_{p}_{ic}_{hh}")
                    nc.gpsimd.partition_broadcast(bc_sb[:], rec[:], channels=64)
                    bcs.append(bc_sb)
                for hh in range(2):
                    head = 2 * p + hh
                    cof = (head // 2) * T
                    pof = (head % 2) * 64
                    dst = attnT[pof:pof + 64, cof + ic * 512:cof + (ic + 1) * 512]
                    nc.vector.tensor_mul(dst, av[hh][0:64, :], bcs[hh][:])

            def out_proj_chunk(tcc, dma_eng=None, split=False):
                """output projection for token chunk tcc.  split=True (tail):
                per-oc eviction on alternating engines + per-oc DMA on two
                queues so the last chunks drain with minimal serial chain."""
                tsl = slice(tcc * 128, (tcc + 1) * 128)
                po_sb = p1t.tile([128, 1024], BF16, tag="po_sb", name=f"po_sb_{tcc}")
                for oc in range(2):
                    po = pj.tile([128, 512], F32, tag="pj", name=f"po_{tcc}_{oc}")
                    for cc in range(2):
                        nc.tensor.matmul(
                            po[:],
                            attnT[:, cc * T + tcc * 128:cc * T + (tcc + 1) * 128],
                            wout_sb[:, cc * D + oc * 512:cc * D + (oc + 1) * 512],
                            start=(cc == 0), stop=(cc == 1),
                        )
                    osl = slice(oc * 512, (oc + 1) * 512)
                    if split:
                        ev = (nc.vector.tensor_copy, nc.scalar.copy)[oc]
                        ev(po_sb[:, osl], po[:])
                        (nc.sync, nc.gpsimd)[oc].dma_start(OUT[tsl, osl], po_sb[:, osl])
                    else:
                        nc.vector.tensor_copy(po_sb[:, osl], po[:])
                if not split:
                    (dma_eng or nc.sync).dma_start(OUT[tsl, :], po_sb[:])

            # ---------------- schedule: n-major waves -------------------
            # Wave n runs attention for query chunk n of both head pairs.
            # exp on ScalarE outweighs the attention matmuls 4:3, so each
            # wave carries PE filler: waves 0-2 the next wave's qk/v
            # projections, wave 3 the whole output projection (its waves'
            # attnT rows are complete by then).  Wave 0 starts as soon as
            # its own q/k chunks exist (m0/m1); its scores+exp are issued
            # before the v projections so ScalarE spins up ASAP.
            def load_wout():
                nc.scalar.dma_start(wout_sb[:, 0:D], WOUT[0:128, :])
                nc.scalar.dma_start(wout_sb[:, D:2 * D], WOUT[128:256, :])

            qk_proj_chunk(0, 0)
            qk_proj_chunk(1, 0)
            attn_ic(0, 0,
                    fillers=[lambda: qk_proj_chunk(2, 0), lambda: qk_proj_chunk(3, 0),
                             lambda: qk_proj_chunk(0, 1), lambda: qk_proj_chunk(1, 1)],
                    mid=lambda: [v_proj_chunk(t) for t in range(4)])
            attn_ic(1, 0,
                    fillers=[lambda: qk_proj_chunk(2, 1), lambda: qk_proj_chunk(3, 1),
                             lambda: v_proj_chunk(4), lambda: v_proj_chunk(5),
                             lambda: v_proj_chunk(6), lambda: v_proj_chunk(7),
                             lambda: load_x_block(2), load_wout])
            for n in range(1, TC512):
                fill0, fill1 = [], []
                if n < 3:
                    nx = n + 1
                    if nx + 1 < TC512:
                        fill0 += [lambda b=nx + 1: load_x_block(b)]
                    fill0 += [(lambda m=m: qk_proj_chunk(m, nx)) for m in range(4)]
                    fill0 += [(lambda t=t: v_proj_chunk(t)) for t in range(4 * nx, 4 * nx + 4)]
                if n == 3:
                    fill1 += [(lambda t=t, e=e: out_proj_chunk(t, e))
                              for t, e in zip(range(0, 12),
                                              [nc.sync, nc.gpsimd] * 6)]
                half = len(fill0) // 2
                attn_ic(0, n, fill0[:half] + fill1[:8])
                attn_ic(1, n, fill0[half:] + fill1[8:])
            for tcc in range(12, 16):
                out_proj_chunk(tcc, split=True)

            if debug:
                nc.sync.dma_start(DBG_QKT[:], qkT[:])
                nc.sync.dma_start(DBG_V[:], v_sb[:])
                nc.sync.dma_start(DBG_ATT[:], attnT[:])

    nc.compile()
    return nc


_DEINT = list(range(0, DK, 2)) + list(range(1, DK, 2))


def _rope_tables():
    j = np.arange(DK // 2, dtype=np.float64)
    inv_freq = THETA ** (-2.0 * j / DK)
    t = np.arange(T, dtype=np.float64)
    ang = t[None, :] * inv_freq[:, None]          # [32, T]
    ang = np.tile(ang, (4, 1))                    # [128, T]
    return np.cos(ang), np.sin(ang)


def _psw():
    M = np.zeros((128, 128), dtype=np.float32)
    for p in range(128):
        pm = p % 64
        if pm < 32:
            M[p, p + 32] = -1.0
        else:
            M[p, p - 32] = 1.0
    return np.ascontiguousarray(M.T)


def shard_inputs(x, Wqkv, bqkv, Wout, bout):
    bf = ml_dtypes.bfloat16
    x = np.asarray(x, dtype=np.float32)
    Wqkv = np.asarray(Wqkv, dtype=np.float32)
    bqkv = np.asarray(bqkv, dtype=np.float32)
    Wout = np.asarray(Wout, dtype=np.float32)

    cos_t, sin_t = _rope_tables()
    csq = np.ascontiguousarray(
        np.concatenate([cos_t, sin_t], axis=1)).astype(bf)   # [128, 2T]
    psw = _psw()
    # stationary strict-upper mask (lhsT[c,p] = 1 for key p > query c) and
    # the -800*I moving operand for the diagonal-block mask matmul
    triu = np.triu(np.ones((128, 128), dtype=np.float32), 1)
    negi = MASK_BIAS * np.eye(128, dtype=np.float32)
    pt = np.ascontiguousarray(np.concatenate([psw, triu, negi], axis=1)).astype(bf)
    ones64 = np.ones((1, 64), dtype=np.float32)

    Wfull = np.concatenate([Wqkv, bqkv[:, None]], axis=1)  # [3072, 1025]

    xt = {}
    for b in range(B):
        xt[b] = np.ascontiguousarray(
            np.concatenate([x[b].T, np.ones((1, T), np.float32)], axis=0)
        ).astype(bf)

    in_maps = []
    for c in range(NCORES):
        b = c // 4
        heads = [4 * (c % 4) + i for i in range(HEADS_PER_CORE)]
        # chunk order: [Qp0 | Kp0 | Qp1 | Kp1], each 128 rows (2 heads x 64)
        qk_rows = []
        for p in range(2):
            qrows, krows = [], []
            for h in (2 * p, 2 * p + 1):
                H = heads[h]
                qrows += [H * 192 + j for j in _DEINT]
                krows += [H * 192 + 64 + j for j in _DEINT]
            qk_rows += qrows + krows
        v_rows = []
        for h in range(4):
            H = heads[h]
            v_rows += [H * 192 + 128 + j for j in range(DK)]
        vch_out = []
        for h in range(4):
            H = heads[h]
            vch_out += [H * 64 + j for j in range(DK)]

        in_maps.append({
            "XT": xt[b],
            "WQK": np.ascontiguousarray(Wfull[qk_rows].T).astype(bf),
            "WV": np.ascontiguousarray(Wfull[v_rows].T).astype(bf),
            "WOUT": np.ascontiguousarray(Wout[:, vch_out].T).astype(bf),
            "PT": pt,
            "CSQ": csq,
            "ONES64": ones64,
        })
    return in_maps


_CACHED = {}


def _get_program(with_bias=False, debug=False):
    key = (bool(with_bias), bool(debug))
    if key not in _CACHED:
        _CACHED[key] = build_program(with_bias=with_bias, debug=debug)
    return _CACHED[key]


def run_cores(inputs, debug=False, trace=False, tmpdir=None):
    with_bias = bool(np.any(np.asarray(inputs["bqkv"], dtype=np.float32)))
    nc = _get_program(with_bias=with_bias, debug=debug)
    in_maps = shard_inputs(**inputs)
    res = run_bass_kernel_spmd(
        nc, in_maps, core_ids=list(range(NCORES)), trace=trace, tmpdir=tmpdir,
    )
    return res


def combine(results, bout):
    bout = np.asarray(bout, dtype=np.float32)
    out = np.empty((B, T, D), dtype=np.float32)
    for b in range(B):
        acc = results[4 * b]["OUT"].astype(np.float32)
        for c in range(4 * b + 1, 4 * b + 4):
            acc = acc + results[c]["OUT"].astype(np.float32)
        out[b] = acc + bout[None, :]
    return out


def kernel(x, Wqkv, bqkv, Wout, bout):
    res = run_cores(dict(x=x, Wqkv=Wqkv, bqkv=bqkv, Wout=Wout, bout=bout))
    return combine(res.results, bout)
